# revision 18
# baseline (speedup 1.0000x reference)
"""Multi-head attention (B=2, N=4096, C=512, H=8) on 8 trn2 NeuronCores — v9.

Sharding: core -> (batch b = core//4, head-pair hp = core%4), data parallel
over B and tensor parallel over the 8 heads (2 heads per core), with
column-sharded qkv weights. Each core returns, per (query-group g, head h),
the UNNORMALIZED attention output transposed [64, 512] plus a denominator
row (65 rows total, f16). The host divides by the denominators, assembles
[B, 4096, 512] and applies the output projection (proj_w/proj_b) there.

Per-core device kernel:
  xsb [128, 4, 4096] f16    <- whole x[b]^T resident in SBUF
  vsb [128 keys, 32 m, 130] <- x^T^T @ wv (+bias) per m-tile, ones col at 64
  qT/kT [128, 4096] f16     <- wqk^T @ x^T (+bias), rows 0:64 h0 / 64:128 h1
  per (g of 8 query groups, chunk of 3 key m-tiles):
    scores emitted as h0/h1 matmul PAIRS on disjoint 64-partition PE row
    groups (contraction=64: h0 rows 0-63, h1 rows 64-127). HW-verified:
    paired row-group matmuls co-execute (~126 ns/MM vs ~255 serial), so
    scores run at full-array throughput despite K=64.
    each head's chunk fills one 3-bank psum tile [128, 3, 512]; ONE wide
    exp op per head drains it: ACT native Exp for one head, DVE
    Schraudolph (i16 bits bitcast to f16) for the other, alternating by
    chunk so each head is half-exact / half-approx and both engines stay
    ~60% loaded under the tensor engine.
    av_h [65, 512] PSUM += vsb[:, m, 65h:65h+65] x et_h per m (row 64 =
    softmax denominator via the ones column); consecutive same-bank AV
    matmuls keep the PE at its ~216 ns/512-col streaming floor.
  per (g, h): asb = f16(av) on ACT, DMA'd to av_d rows [(2g+h)*65 : +65].
No projection and no normalization on device; psum = 2x3-bank score tiles
+ 2 AV banks = 8 banks exactly.
"""

import numpy as np

_state = {}

B, N, C, H, DH = 2, 4096, 512, 8, 64
SCALE = DH ** -0.5
GQ = 512          # queries per group
NG = N // GQ      # 8 groups
MT = N // 128     # 32 key m-tiles
LOG2E = 1.4426950408889634
A16 = 1024.0 * LOG2E * SCALE
B16 = 1024.0 * 15.0 - 46.0


def _build_nc(debug=False):
    from contextlib import ExitStack

    import concourse.bacc as bacc
    import concourse.tile as tile
    from concourse import mybir

    f16 = mybir.dt.float16
    f32 = mybir.dt.float32
    i16 = mybir.dt.int16
    EXP = mybir.ActivationFunctionType.Exp
    COPY = mybir.ActivationFunctionType.Copy
    MULT = mybir.AluOpType.mult
    ADD = mybir.AluOpType.add

    nc = bacc.Bacc(None, target_bir_lowering=False)
    with tile.TileContext(nc) as tc, ExitStack() as ctx:
        dram = ctx.enter_context(tc.tile_pool(name="dram", bufs=1, space="DRAM"))
        xt_d = dram.tile([C, N], f16, kind="ExternalInput", name="xt",
                         uniquify=False, tag="dxt")
        wqk_d = dram.tile([C, 256], f16, kind="ExternalInput", name="wqk",
                          uniquify=False, tag="dwqk")
        bqk_d = dram.tile([128, 2], f32, kind="ExternalInput", name="bqk",
                          uniquify=False, tag="dbqk")
        wv_d = dram.tile([C, 128], f16, kind="ExternalInput", name="wv",
                         uniquify=False, tag="dwv")
        bv_d = dram.tile([128, 128], f16, kind="ExternalInput", name="bv",
                         uniquify=False, tag="dbv")
        av_d = dram.tile([2 * NG * 65, GQ], f16, kind="ExternalOutput",
                         name="av", uniquify=False, tag="dav")

        const = ctx.enter_context(tc.tile_pool(name="const", bufs=1))
        wqk_sb = const.tile([128, 4, 256], f16, name="wqk_sb", tag="wqk_sb")
        nc.gpsimd.dma_start(wqk_sb[:], wqk_d.rearrange("(k p) f -> p k f", p=128))
        wv_sb = const.tile([128, 4, 128], f16, name="wv_sb", tag="wv_sb")
        nc.gpsimd.dma_start(wv_sb[:], wv_d.rearrange("(k p) f -> p k f", p=128))
        bqk_sb = const.tile([128, 2], f32, name="bqk_sb", tag="bqk_sb")
        nc.gpsimd.dma_start(bqk_sb[:], bqk_d[:])
        bv_sb = const.tile([128, 128], f16, name="bv_sb", tag="bv_sb")
        nc.gpsimd.dma_start(bv_sb[:], bv_d[:])

        persist = ctx.enter_context(tc.tile_pool(name="persist", bufs=1))
        xsb = persist.tile([128, 4, N], f16, name="xsb", tag="xsb")
        xt_r = xt_d.rearrange("(k p) n -> p k n", p=128)
        for blk in range(NG):
            nc.sync.dma_start(xsb[:, :, GQ * blk:GQ * (blk + 1)],
                              xt_r[:, :, GQ * blk:GQ * (blk + 1)])
        qT = persist.tile([128, N], f16, name="qT", tag="qT")
        kT = persist.tile([128, N], f16, name="kT", tag="kT")
        vsb = persist.tile([128, MT, 130], f16, name="vsb", tag="vsb")
        vones = vsb.rearrange("p m (a b) -> p m a b", a=2)
        nc.vector.memset(vones[:, :, 0, 64:65], 1.0)
        nc.vector.memset(vones[:, :, 1, 64:65], 1.0)

        # psum budget: spool 3x2 banks + apool 2 banks = 8
        spool = ctx.enter_context(tc.tile_pool(name="sp", bufs=3, space="PSUM"))
        apool = ctx.enter_context(tc.tile_pool(name="ap", bufs=2, space="PSUM"))
        epool = ctx.enter_context(tc.tile_pool(name="ep", bufs=8))
        rpool = ctx.enter_context(tc.tile_pool(name="rp", bufs=2))

        def emit_v(m):
            vt = spool.tile([128, 2, 512], f32, name="vp", tag="sch")
            vp = vt[:, 0, :]
            for k in range(4):
                nc.tensor.matmul(vp[:, 0:128],
                                 xsb[:, k, 128 * m:128 * (m + 1)],
                                 wv_sb[:, k, :],
                                 start=(k == 0), stop=(k == 3))
            src = vp[:, 0:128].rearrange("p (a b) -> p a b", a=2)
            dst = vsb[:, m, :].rearrange("p (a b) -> p a b", a=2)
            bvv = bv_sb.rearrange("p (a b) -> p a b", a=2)
            nc.vector.tensor_add(dst[:, :, 0:64], src, bvv)

        def emit_qk(g):
            qt = spool.tile([128, 2, 512], f32, name="qp", tag="sch")
            qp = qt[:, 0, :]
            for k in range(4):
                nc.tensor.matmul(qp, wqk_sb[:, k, 0:128],
                                 xsb[:, k, GQ * g:GQ * (g + 1)],
                                 start=(k == 0), stop=(k == 3))
            kt = spool.tile([128, 2, 512], f32, name="kp", tag="sch")
            kp = kt[:, 0, :]
            for k in range(4):
                nc.tensor.matmul(kp, wqk_sb[:, k, 128:256],
                                 xsb[:, k, GQ * g:GQ * (g + 1)],
                                 start=(k == 0), stop=(k == 3))
            nc.vector.tensor_scalar_add(qT[:, GQ * g:GQ * (g + 1)],
                                        qp, bqk_sb[:, 0:1])
            nc.vector.tensor_scalar_add(kT[:, GQ * g:GQ * (g + 1)],
                                        kp, bqk_sb[:, 1:2])

        av_tiles = {}

        def emit_post(g, h):
            a = av_tiles.pop((g, h))
            asb = rpool.tile([65, 512], f16, name="asb", tag="asb")
            nc.scalar.activation(asb[:], a[0:65, :], COPY)
            r0 = (2 * g + h) * 65
            nc.sync.dma_start(av_d[r0:r0 + 65, :], asb[:])

        def emit_av(g, ms, et0, et1):
            if (g, 0) not in av_tiles:
                av_tiles[(g, 0)] = apool.tile([128, 512], f32, name="av0",
                                              tag="av")
                av_tiles[(g, 1)] = apool.tile([128, 512], f32, name="av1",
                                              tag="av")
            for h, et in ((0, et0), (1, et1)):
                a = av_tiles[(g, h)]
                for j, m in enumerate(ms):
                    nc.tensor.matmul(a[0:65, :],
                                     vsb[:, m, 65 * h:65 * h + 65],
                                     et[:, 512 * j:512 * (j + 1)],
                                     start=(m == 0), stop=(m == MT - 1),
                                     skip_group_check=True)
            if ms[-1] == MT - 1:
                emit_post(g, 0)
                emit_post(g, 1)

        # ---- preamble: ALL q/k/v materialized (scores for any group read
        # keys from every m-tile, so kT must be complete before group 0) ----
        for gg in range(NG):
            emit_qk(gg)
            for t in range(4):
                emit_v(4 * gg + t)

        CH = 2
        chunks = [list(range(c, min(c + CH, MT))) for c in range(0, MT, CH)]
        SKEW = 3
        inflight = []
        ci = 0
        for g in range(NG):
            for ms in chunks:
                # scores emitted as h0/h1 pairs on disjoint 64-row PE groups
                # (they co-execute, ~2x); each head's chunk fills a 3-bank
                # psum tile drained by ONE wide exp op per head
                st0 = spool.tile([128, 2, 512], f32, name="st0", tag="sch")
                st1 = spool.tile([128, 2, 512], f32, name="st1", tag="sch")
                et0 = epool.tile([128, 1024], f16, name="et0", tag="et")
                et1 = epool.tile([128, 1024], f16, name="et1", tag="et")
                sa, sv = (st0, st1) if ci % 2 == 0 else (st1, st0)
                ea, ev = (et0, et1) if ci % 2 == 0 else (et1, et0)
                # exp per m-plane right behind its scores pair, so each
                # psum plane drains ~one pair earlier (shrinks the
                # st-tile-reuse latency chain)
                for j, m in enumerate(ms):
                    nc.tensor.matmul(st0[:, j, :],
                                     kT[0:64, 128 * m:128 * (m + 1)],
                                     qT[0:64, GQ * g:GQ * (g + 1)],
                                     start=True, stop=True)
                    nc.tensor.matmul(st1[:, j, :],
                                     kT[64:128, 128 * m:128 * (m + 1)],
                                     qT[64:128, GQ * g:GQ * (g + 1)],
                                     start=True, stop=True)
                    jj = slice(512 * j, 512 * (j + 1))
                    nc.scalar.activation(ea[:, jj], sa[:, j, :],
                                         EXP, scale=SCALE)
                    nc.vector.tensor_scalar(ev[:, jj].bitcast(i16),
                                            sv[:, j, :],
                                            A16, B16, MULT, ADD)
                ci += 1
                inflight.append((g, ms, et0, et1))
                if len(inflight) > SKEW:
                    emit_av(*inflight.pop(0))
        while inflight:
            emit_av(*inflight.pop(0))

    nc.compile()
    return nc


def _get_nc():
    if "nc" not in _state:
        _state["nc"] = _build_nc()
    return _state["nc"]


def _make_in_maps(x, qkv_w, qkv_b):
    f16 = np.float16
    x = np.asarray(x, np.float32)
    qkv_w = np.asarray(qkv_w, np.float32)
    qkv_b = np.asarray(qkv_b, np.float32)
    in_maps = []
    for core in range(8):
        b, hp = divmod(core, 4)
        h0, h1 = 2 * hp, 2 * hp + 1
        xt = np.ascontiguousarray(x[b].T).astype(f16)
        rq = np.concatenate([qkv_w[64 * h0:64 * h0 + 64],
                             qkv_w[64 * h1:64 * h1 + 64]], 0)
        rk = np.concatenate([qkv_w[C + 64 * h0:C + 64 * h0 + 64],
                             qkv_w[C + 64 * h1:C + 64 * h1 + 64]], 0)
        wqk = np.ascontiguousarray(np.concatenate([rq, rk], 0).T).astype(f16)
        bq = np.concatenate([qkv_b[64 * h0:64 * h0 + 64],
                             qkv_b[64 * h1:64 * h1 + 64]])
        bk = np.concatenate([qkv_b[C + 64 * h0:C + 64 * h0 + 64],
                             qkv_b[C + 64 * h1:C + 64 * h1 + 64]])
        bqk = np.ascontiguousarray(np.stack([bq, bk], 1)).astype(np.float32)
        rv = np.concatenate([qkv_w[2 * C + 64 * h0:2 * C + 64 * h0 + 64],
                             qkv_w[2 * C + 64 * h1:2 * C + 64 * h1 + 64]], 0)
        wv = np.ascontiguousarray(rv.T).astype(f16)
        bvrow = np.concatenate([qkv_b[2 * C + 64 * h0:2 * C + 64 * h0 + 64],
                                qkv_b[2 * C + 64 * h1:2 * C + 64 * h1 + 64]])
        bv = np.ascontiguousarray(
            np.broadcast_to(bvrow[None, :], (128, 128))).astype(f16)
        in_maps.append(dict(xt=xt, wqk=wqk, bqk=bqk, wv=wv, bv=bv))
    return in_maps


def _gather(results, proj_w, proj_b):
    proj_w = np.asarray(proj_w, np.float32)
    proj_b = np.asarray(proj_b, np.float32)
    out = np.empty((B, N, C), np.float32)
    for b in range(B):
        Xb = np.empty((N, C), np.float32)
        for hp in range(4):
            av = results[4 * b + hp]["av"].astype(np.float32)
            av = av.reshape(NG, 2, 65, GQ)
            for lh in range(2):
                att = av[:, lh, 0:64, :] / av[:, lh, 64:65, :]  # [NG,64,GQ]
                att = att.transpose(0, 2, 1).reshape(N, 64)
                Xb[:, 128 * hp + 64 * lh:128 * hp + 64 * lh + 64] = att
        out[b] = Xb @ proj_w.T + proj_b
    return out


def _run(x, qkv_w, qkv_b, proj_w, proj_b, trace=False, tmpdir=None):
    from concourse import bass_utils
    nc = _get_nc()
    in_maps = _make_in_maps(x, qkv_w, qkv_b)
    res = bass_utils.run_bass_kernel_spmd(
        nc, in_maps, core_ids=list(range(8)), trace=trace, tmpdir=tmpdir)
    return _gather(res.results, proj_w, proj_b), res


def kernel(x, qkv_w, qkv_b, proj_w, proj_b):
    out, _ = _run(x, qkv_w, qkv_b, proj_w, proj_b, trace=False)
    return out


# revision 20
# speedup vs baseline: 1.0503x; 1.0503x over previous
"""Multi-head attention (B=2, N=4096, C=512, H=8) on 8 trn2 NeuronCores — v9.

Sharding: core -> (batch b = core//4, head-pair hp = core%4), data parallel
over B and tensor parallel over the 8 heads (2 heads per core), with
column-sharded qkv weights. Each core returns, per (query-group g, head h),
the UNNORMALIZED attention output transposed [64, 512] plus a denominator
row (65 rows total, f16). The host divides by the denominators, assembles
[B, 4096, 512] and applies the output projection (proj_w/proj_b) there.

Per-core device kernel:
  xsb [128, 4, 4096] f16    <- whole x[b]^T resident in SBUF
  vsb [128 keys, 32 m, 130] <- x^T^T @ wv (+bias) per m-tile, ones col at 64
  qT/kT [128, 4096] f16     <- wqk^T @ x^T (+bias), rows 0:64 h0 / 64:128 h1
  per (g of 8 query groups, chunk of 2 key m-tiles):
    scores emitted as h0/h1 matmul PAIRS on disjoint 64-partition PE row
    groups (contraction=64: h0 rows 0-63, h1 rows 64-127). HW-verified:
    paired row-group matmuls co-execute (~126 ns/MM vs ~255 serial), so
    scores run at full-array throughput despite K=64.
    each head's chunk fills one 2-bank psum tile [128, 2, 512]; ONE wide
    exp op per head drains it: ACT native Exp for one head, DVE
    Schraudolph (i16 bits bitcast to f16) for the other, alternating by
    chunk so each head is half-exact / half-approx and both engines stay
    ~60% loaded under the tensor engine.
    av_h [65, 512] PSUM += vsb[:, m, 65h:65h+65] x et_h per m (row 64 =
    softmax denominator via the ones column); consecutive same-bank AV
    matmuls keep the PE at its ~216 ns/512-col streaming floor.
  per (g, h): asb = f16(av) on ACT, DMA'd to av_d rows [(2g+h)*65 : +65].
No projection and no normalization on device; psum = 3x2-bank score tiles
+ 2 AV banks = 8 banks exactly (3 score bufs decouple the scores->exp->
scores psum-reuse latency chain by a full chunk).
"""

import numpy as np

_state = {}

B, N, C, H, DH = 2, 4096, 512, 8, 64
SCALE = DH ** -0.5
GQ = 512          # queries per group
NG = N // GQ      # 8 groups
MT = N // 128     # 32 key m-tiles
LOG2E = 1.4426950408889634
A16 = 1024.0 * LOG2E * SCALE
B16 = 1024.0 * 15.0 - 46.0


def _build_nc(debug=False):
    from contextlib import ExitStack

    import concourse.bacc as bacc
    import concourse.tile as tile
    from concourse import mybir

    f16 = mybir.dt.float16
    f32 = mybir.dt.float32
    i16 = mybir.dt.int16
    EXP = mybir.ActivationFunctionType.Exp
    COPY = mybir.ActivationFunctionType.Copy
    MULT = mybir.AluOpType.mult
    ADD = mybir.AluOpType.add

    nc = bacc.Bacc(None, target_bir_lowering=False)
    with tile.TileContext(nc) as tc, ExitStack() as ctx:
        dram = ctx.enter_context(tc.tile_pool(name="dram", bufs=1, space="DRAM"))
        xt_d = dram.tile([C, N], f16, kind="ExternalInput", name="xt",
                         uniquify=False, tag="dxt")
        wqk_d = dram.tile([C, 256], f16, kind="ExternalInput", name="wqk",
                          uniquify=False, tag="dwqk")
        bqk_d = dram.tile([128, 2], f32, kind="ExternalInput", name="bqk",
                          uniquify=False, tag="dbqk")
        wv_d = dram.tile([C, 128], f16, kind="ExternalInput", name="wv",
                         uniquify=False, tag="dwv")
        bv_d = dram.tile([128, 128], f16, kind="ExternalInput", name="bv",
                         uniquify=False, tag="dbv")
        av_d = dram.tile([2 * NG * 65, GQ], f16, kind="ExternalOutput",
                         name="av", uniquify=False, tag="dav")

        const = ctx.enter_context(tc.tile_pool(name="const", bufs=1))
        wqk_sb = const.tile([128, 4, 256], f16, name="wqk_sb", tag="wqk_sb")
        nc.gpsimd.dma_start(wqk_sb[:], wqk_d.rearrange("(k p) f -> p k f", p=128))
        wv_sb = const.tile([128, 4, 128], f16, name="wv_sb", tag="wv_sb")
        nc.gpsimd.dma_start(wv_sb[:], wv_d.rearrange("(k p) f -> p k f", p=128))
        bqk_sb = const.tile([128, 2], f32, name="bqk_sb", tag="bqk_sb")
        nc.gpsimd.dma_start(bqk_sb[:], bqk_d[:])
        bv_sb = const.tile([128, 128], f16, name="bv_sb", tag="bv_sb")
        nc.gpsimd.dma_start(bv_sb[:], bv_d[:])

        persist = ctx.enter_context(tc.tile_pool(name="persist", bufs=1))
        xsb = persist.tile([128, 4, N], f16, name="xsb", tag="xsb")
        xt_r = xt_d.rearrange("(k p) n -> p k n", p=128)
        for blk in range(NG):
            nc.sync.dma_start(xsb[:, :, GQ * blk:GQ * (blk + 1)],
                              xt_r[:, :, GQ * blk:GQ * (blk + 1)])
        qT = persist.tile([128, N], f16, name="qT", tag="qT")
        kT = persist.tile([128, N], f16, name="kT", tag="kT")
        vsb = persist.tile([128, MT, 130], f16, name="vsb", tag="vsb")
        vones = vsb.rearrange("p m (a b) -> p m a b", a=2)
        nc.vector.memset(vones[:, :, 0, 64:65], 1.0)
        nc.vector.memset(vones[:, :, 1, 64:65], 1.0)

        # psum budget: spool 3x2 banks + apool 2 banks = 8
        spool = ctx.enter_context(tc.tile_pool(name="sp", bufs=3, space="PSUM"))
        apool = ctx.enter_context(tc.tile_pool(name="ap", bufs=2, space="PSUM"))
        epool = ctx.enter_context(tc.tile_pool(name="ep", bufs=8))
        rpool = ctx.enter_context(tc.tile_pool(name="rp", bufs=2))

        def emit_v(m):
            vt = spool.tile([128, 2, 512], f32, name="vp", tag="sch")
            vp = vt[:, 0, :]
            for k in range(4):
                nc.tensor.matmul(vp[:, 0:128],
                                 xsb[:, k, 128 * m:128 * (m + 1)],
                                 wv_sb[:, k, :],
                                 start=(k == 0), stop=(k == 3))
            src = vp[:, 0:128].rearrange("p (a b) -> p a b", a=2)
            dst = vsb[:, m, :].rearrange("p (a b) -> p a b", a=2)
            bvv = bv_sb.rearrange("p (a b) -> p a b", a=2)
            nc.vector.tensor_add(dst[:, :, 0:64], src, bvv)

        def emit_qk(g):
            qt = spool.tile([128, 2, 512], f32, name="qp", tag="sch")
            qp = qt[:, 0, :]
            for k in range(4):
                nc.tensor.matmul(qp, wqk_sb[:, k, 0:128],
                                 xsb[:, k, GQ * g:GQ * (g + 1)],
                                 start=(k == 0), stop=(k == 3))
            kt = spool.tile([128, 2, 512], f32, name="kp", tag="sch")
            kp = kt[:, 0, :]
            for k in range(4):
                nc.tensor.matmul(kp, wqk_sb[:, k, 128:256],
                                 xsb[:, k, GQ * g:GQ * (g + 1)],
                                 start=(k == 0), stop=(k == 3))
            nc.vector.tensor_scalar_add(qT[:, GQ * g:GQ * (g + 1)],
                                        qp, bqk_sb[:, 0:1])
            nc.vector.tensor_scalar_add(kT[:, GQ * g:GQ * (g + 1)],
                                        kp, bqk_sb[:, 1:2])

        av_tiles = {}

        def emit_post(g, h):
            a = av_tiles.pop((g, h))
            asb = rpool.tile([65, 512], f16, name="asb", tag="asb")
            nc.scalar.activation(asb[:], a[0:65, :], COPY)
            r0 = (2 * g + h) * 65
            nc.sync.dma_start(av_d[r0:r0 + 65, :], asb[:])

        def emit_av(g, ms, et0, et1):
            if (g, 0) not in av_tiles:
                av_tiles[(g, 0)] = apool.tile([128, 512], f32, name="av0",
                                              tag="av")
                av_tiles[(g, 1)] = apool.tile([128, 512], f32, name="av1",
                                              tag="av")
            for h, et in ((0, et0), (1, et1)):
                a = av_tiles[(g, h)]
                for j, m in enumerate(ms):
                    nc.tensor.matmul(a[0:65, :],
                                     vsb[:, m, 65 * h:65 * h + 65],
                                     et[:, 512 * j:512 * (j + 1)],
                                     start=(m == 0), stop=(m == MT - 1),
                                     skip_group_check=True)
            if ms[-1] == MT - 1:
                emit_post(g, 0)
                emit_post(g, 1)

        # ---- preamble: ALL q/k/v materialized (scores for any group read
        # keys from every m-tile, so kT must be complete before group 0) ----
        for gg in range(NG):
            emit_qk(gg)
            for t in range(4):
                emit_v(4 * gg + t)

        CH = 2
        chunks = [list(range(c, min(c + CH, MT))) for c in range(0, MT, CH)]
        SKEW = 3
        inflight = []
        ci = 0
        for g in range(NG):
            for ms in chunks:
                # scores emitted as h0/h1 pairs on disjoint 64-row PE groups
                # (they co-execute, ~2x); each head's chunk fills a 3-bank
                # psum tile drained by ONE wide exp op per head
                st0 = spool.tile([128, 2, 512], f32, name="st0", tag="sch")
                st1 = spool.tile([128, 2, 512], f32, name="st1", tag="sch")
                et0 = epool.tile([128, 1024], f16, name="et0", tag="et")
                et1 = epool.tile([128, 1024], f16, name="et1", tag="et")
                sa, sv = (st0, st1) if ci % 2 == 0 else (st1, st0)
                ea, ev = (et0, et1) if ci % 2 == 0 else (et1, et0)
                for j, m in enumerate(ms):
                    nc.tensor.matmul(st0[:, j, :],
                                     kT[0:64, 128 * m:128 * (m + 1)],
                                     qT[0:64, GQ * g:GQ * (g + 1)],
                                     start=True, stop=True)
                    nc.tensor.matmul(st1[:, j, :],
                                     kT[64:128, 128 * m:128 * (m + 1)],
                                     qT[64:128, GQ * g:GQ * (g + 1)],
                                     start=True, stop=True)
                nm = len(ms)
                w = 512 * nm
                nc.scalar.activation(ea[:, 0:w], sa[:, 0:nm, :],
                                     EXP, scale=SCALE)
                nc.vector.tensor_scalar(ev[:, 0:w].bitcast(i16),
                                        sv[:, 0:nm, :],
                                        A16, B16, MULT, ADD)
                ci += 1
                inflight.append((g, ms, et0, et1))
                if len(inflight) > SKEW:
                    emit_av(*inflight.pop(0))
        while inflight:
            emit_av(*inflight.pop(0))

    nc.compile()
    return nc


def _get_nc():
    if "nc" not in _state:
        _state["nc"] = _build_nc()
    return _state["nc"]


def _make_in_maps(x, qkv_w, qkv_b):
    f16 = np.float16
    x = np.asarray(x, np.float32)
    qkv_w = np.asarray(qkv_w, np.float32)
    qkv_b = np.asarray(qkv_b, np.float32)
    in_maps = []
    for core in range(8):
        b, hp = divmod(core, 4)
        h0, h1 = 2 * hp, 2 * hp + 1
        xt = np.ascontiguousarray(x[b].T).astype(f16)
        rq = np.concatenate([qkv_w[64 * h0:64 * h0 + 64],
                             qkv_w[64 * h1:64 * h1 + 64]], 0)
        rk = np.concatenate([qkv_w[C + 64 * h0:C + 64 * h0 + 64],
                             qkv_w[C + 64 * h1:C + 64 * h1 + 64]], 0)
        wqk = np.ascontiguousarray(np.concatenate([rq, rk], 0).T).astype(f16)
        bq = np.concatenate([qkv_b[64 * h0:64 * h0 + 64],
                             qkv_b[64 * h1:64 * h1 + 64]])
        bk = np.concatenate([qkv_b[C + 64 * h0:C + 64 * h0 + 64],
                             qkv_b[C + 64 * h1:C + 64 * h1 + 64]])
        bqk = np.ascontiguousarray(np.stack([bq, bk], 1)).astype(np.float32)
        rv = np.concatenate([qkv_w[2 * C + 64 * h0:2 * C + 64 * h0 + 64],
                             qkv_w[2 * C + 64 * h1:2 * C + 64 * h1 + 64]], 0)
        wv = np.ascontiguousarray(rv.T).astype(f16)
        bvrow = np.concatenate([qkv_b[2 * C + 64 * h0:2 * C + 64 * h0 + 64],
                                qkv_b[2 * C + 64 * h1:2 * C + 64 * h1 + 64]])
        bv = np.ascontiguousarray(
            np.broadcast_to(bvrow[None, :], (128, 128))).astype(f16)
        in_maps.append(dict(xt=xt, wqk=wqk, bqk=bqk, wv=wv, bv=bv))
    return in_maps


def _gather(results, proj_w, proj_b):
    proj_w = np.asarray(proj_w, np.float32)
    proj_b = np.asarray(proj_b, np.float32)
    out = np.empty((B, N, C), np.float32)
    for b in range(B):
        Xb = np.empty((N, C), np.float32)
        for hp in range(4):
            av = results[4 * b + hp]["av"].astype(np.float32)
            av = av.reshape(NG, 2, 65, GQ)
            for lh in range(2):
                att = av[:, lh, 0:64, :] / av[:, lh, 64:65, :]  # [NG,64,GQ]
                att = att.transpose(0, 2, 1).reshape(N, 64)
                Xb[:, 128 * hp + 64 * lh:128 * hp + 64 * lh + 64] = att
        out[b] = Xb @ proj_w.T + proj_b
    return out


def _run(x, qkv_w, qkv_b, proj_w, proj_b, trace=False, tmpdir=None):
    from concourse import bass_utils
    nc = _get_nc()
    in_maps = _make_in_maps(x, qkv_w, qkv_b)
    res = bass_utils.run_bass_kernel_spmd(
        nc, in_maps, core_ids=list(range(8)), trace=trace, tmpdir=tmpdir)
    return _gather(res.results, proj_w, proj_b), res


def kernel(x, qkv_w, qkv_b, proj_w, proj_b):
    out, _ = _run(x, qkv_w, qkv_b, proj_w, proj_b, trace=False)
    return out


# revision 22
# speedup vs baseline: 1.2622x; 1.2018x over previous
"""Multi-head attention (B=2, N=4096, C=512, H=8) on 8 trn2 NeuronCores — v9.

Sharding: core -> (batch b = core//4, head-pair hp = core%4), data parallel
over B and tensor parallel over the 8 heads (2 heads per core), with
column-sharded qkv weights. Each core returns, per (query-group g, head h),
the UNNORMALIZED attention output transposed [64, 512] plus a denominator
row (65 rows total, f16). The host divides by the denominators, assembles
[B, 4096, 512] and applies the output projection (proj_w/proj_b) there.

Per-core device kernel:
  xsb [128, 4, 4096] f16    <- whole x[b]^T resident in SBUF
  vsb [128 keys, 32 m, 130] <- x^T^T @ wv (+bias) per m-tile, ones col at 64
  qT/kT [128, 4096] f16     <- wqk^T @ x^T (+bias), rows 0:64 h0 / 64:128 h1
  per (g of 8 query groups, chunk of 2 key m-tiles):
    scores emitted as h0/h1 matmul PAIRS on disjoint 64-partition PE row
    groups (contraction=64: h0 rows 0-63, h1 rows 64-127). HW-verified:
    paired row-group matmuls co-execute (~126 ns/MM vs ~255 serial), so
    scores run at full-array throughput despite K=64.
    each head's chunk fills one 2-bank psum tile [128, 2, 512]; ONE wide
    exp op per head drains it: ACT native Exp for one head, DVE
    Schraudolph (i16 bits bitcast to f16) for the other, alternating by
    chunk so each head is half-exact / half-approx and both engines stay
    ~60% loaded under the tensor engine.
    av_h [65, 512] PSUM += vsb[:, m, 65h:65h+65] x et_h per m (row 64 =
    softmax denominator via the ones column); consecutive same-bank AV
    matmuls keep the PE at its ~216 ns/512-col streaming floor.
  per (g, h): asb = f16(av) on ACT, DMA'd to av_d rows [(2g+h)*65 : +65].
No projection and no normalization on device; psum = 3x2-bank score tiles
+ 2 AV banks = 8 banks exactly (3 score bufs decouple the scores->exp->
scores psum-reuse latency chain by a full chunk).
"""

import numpy as np

_state = {}

B, N, C, H, DH = 2, 4096, 512, 8, 64
SCALE = DH ** -0.5
GQ = 512          # queries per group
NG = N // GQ      # 8 groups
MT = N // 128     # 32 key m-tiles
LOG2E = 1.4426950408889634
A16 = 1024.0 * LOG2E * SCALE
B16 = 1024.0 * 15.0 - 46.0


def _build_nc(debug=False):
    from contextlib import ExitStack

    import concourse.bacc as bacc
    import concourse.tile as tile
    from concourse import mybir

    f16 = mybir.dt.float16
    f32 = mybir.dt.float32
    i16 = mybir.dt.int16
    EXP = mybir.ActivationFunctionType.Exp
    COPY = mybir.ActivationFunctionType.Copy
    MULT = mybir.AluOpType.mult
    ADD = mybir.AluOpType.add

    nc = bacc.Bacc(None, target_bir_lowering=False)
    with tile.TileContext(nc) as tc, ExitStack() as ctx:
        dram = ctx.enter_context(tc.tile_pool(name="dram", bufs=1, space="DRAM"))
        xt_d = dram.tile([C, N], f16, kind="ExternalInput", name="xt",
                         uniquify=False, tag="dxt")
        wqk_d = dram.tile([C, 256], f16, kind="ExternalInput", name="wqk",
                          uniquify=False, tag="dwqk")
        bqk_d = dram.tile([128, 2], f32, kind="ExternalInput", name="bqk",
                          uniquify=False, tag="dbqk")
        wv_d = dram.tile([C, 128], f16, kind="ExternalInput", name="wv",
                         uniquify=False, tag="dwv")
        bv_d = dram.tile([128, 128], f16, kind="ExternalInput", name="bv",
                         uniquify=False, tag="dbv")
        av_d = dram.tile([2 * NG * 65, GQ], f16, kind="ExternalOutput",
                         name="av", uniquify=False, tag="dav")

        const = ctx.enter_context(tc.tile_pool(name="const", bufs=1))
        wqk_sb = const.tile([128, 4, 256], f16, name="wqk_sb", tag="wqk_sb")
        nc.gpsimd.dma_start(wqk_sb[:], wqk_d.rearrange("(k p) f -> p k f", p=128))
        wv_sb = const.tile([128, 4, 128], f16, name="wv_sb", tag="wv_sb")
        nc.gpsimd.dma_start(wv_sb[:], wv_d.rearrange("(k p) f -> p k f", p=128))
        bqk_sb = const.tile([128, 2], f32, name="bqk_sb", tag="bqk_sb")
        nc.gpsimd.dma_start(bqk_sb[:], bqk_d[:])
        bv_sb = const.tile([128, 128], f16, name="bv_sb", tag="bv_sb")
        nc.gpsimd.dma_start(bv_sb[:], bv_d[:])

        persist = ctx.enter_context(tc.tile_pool(name="persist", bufs=1))
        xsb = persist.tile([128, 4, N], f16, name="xsb", tag="xsb")
        xt_r = xt_d.rearrange("(k p) n -> p k n", p=128)
        for blk in range(NG):
            nc.sync.dma_start(xsb[:, :, GQ * blk:GQ * (blk + 1)],
                              xt_r[:, :, GQ * blk:GQ * (blk + 1)])
        qT = persist.tile([128, N], f16, name="qT", tag="qT")
        kT = persist.tile([128, N], f16, name="kT", tag="kT")
        vsb = persist.tile([128, MT, 130], f16, name="vsb", tag="vsb")
        vones = vsb.rearrange("p m (a b) -> p m a b", a=2)
        nc.vector.memset(vones[:, :, 0, 64:65], 1.0)
        nc.vector.memset(vones[:, :, 1, 64:65], 1.0)

        # psum budget: spool 3x2 banks + apool 2 banks = 8
        spool = ctx.enter_context(tc.tile_pool(name="sp", bufs=3, space="PSUM"))
        apool = ctx.enter_context(tc.tile_pool(name="ap", bufs=2, space="PSUM"))
        epool = ctx.enter_context(tc.tile_pool(name="ep", bufs=8))
        rpool = ctx.enter_context(tc.tile_pool(name="rp", bufs=2))

        def emit_v(m):
            vt = spool.tile([128, 2, 512], f32, name="vp", tag="sch")
            vp = vt[:, 0, :]
            for k in range(4):
                nc.tensor.matmul(vp[:, 0:128],
                                 xsb[:, k, 128 * m:128 * (m + 1)],
                                 wv_sb[:, k, :],
                                 start=(k == 0), stop=(k == 3))
            src = vp[:, 0:128].rearrange("p (a b) -> p a b", a=2)
            dst = vsb[:, m, :].rearrange("p (a b) -> p a b", a=2)
            bvv = bv_sb.rearrange("p (a b) -> p a b", a=2)
            nc.vector.tensor_add(dst[:, :, 0:64], src, bvv)

        def emit_qk(g):
            qt = spool.tile([128, 2, 512], f32, name="qp", tag="sch")
            qp = qt[:, 0, :]
            for k in range(4):
                nc.tensor.matmul(qp, wqk_sb[:, k, 0:128],
                                 xsb[:, k, GQ * g:GQ * (g + 1)],
                                 start=(k == 0), stop=(k == 3))
            kt = spool.tile([128, 2, 512], f32, name="kp", tag="sch")
            kp = kt[:, 0, :]
            for k in range(4):
                nc.tensor.matmul(kp, wqk_sb[:, k, 128:256],
                                 xsb[:, k, GQ * g:GQ * (g + 1)],
                                 start=(k == 0), stop=(k == 3))
            nc.vector.tensor_scalar_add(qT[:, GQ * g:GQ * (g + 1)],
                                        qp, bqk_sb[:, 0:1])
            nc.vector.tensor_scalar_add(kT[:, GQ * g:GQ * (g + 1)],
                                        kp, bqk_sb[:, 1:2])

        av_tiles = {}

        def emit_post(g, h):
            a = av_tiles.pop((g, h))
            asb = rpool.tile([65, 512], f16, name="asb", tag="asb")
            nc.scalar.activation(asb[:], a[0:65, :], COPY)
            r0 = (2 * g + h) * 65
            nc.sync.dma_start(av_d[r0:r0 + 65, :], asb[:])

        def emit_av(g, ms, et0, et1):
            if (g, 0) not in av_tiles:
                av_tiles[(g, 0)] = apool.tile([128, 512], f32, name="av0",
                                              tag="av")
                av_tiles[(g, 1)] = apool.tile([128, 512], f32, name="av1",
                                              tag="av")
            for h, et in ((0, et0), (1, et1)):
                a = av_tiles[(g, h)]
                for j, m in enumerate(ms):
                    nc.tensor.matmul(a[0:65, :],
                                     vsb[:, m, 65 * h:65 * h + 65],
                                     et[:, 512 * j:512 * (j + 1)],
                                     start=(m == 0), stop=(m == MT - 1),
                                     skip_group_check=True)
            if ms[-1] == MT - 1:
                emit_post(g, 0)
                emit_post(g, 1)

        # ---- preamble: ALL q/k/v materialized (scores for any group read
        # keys from every m-tile, so kT must be complete before group 0) ----
        for gg in range(NG):
            emit_qk(gg)
            for t in range(4):
                emit_v(4 * gg + t)

        CH = 2
        chunks = [list(range(c, min(c + CH, MT))) for c in range(0, MT, CH)]
        SKEW = 3
        inflight = []
        ci = 0
        for g in range(NG):
            for ms in chunks:
                # AV of the lagged chunk goes FIRST: the PE executes its
                # queue in order, so ready AV matmuls must not sit behind
                # scores matmuls that are still waiting on psum-tile reuse
                if len(inflight) > SKEW:
                    emit_av(*inflight.pop(0))
                # scores emitted as h0/h1 pairs on disjoint 64-row PE groups
                # (they co-execute, ~2x); each head's chunk fills a 3-bank
                # psum tile drained by ONE wide exp op per head
                st0 = spool.tile([128, 2, 512], f32, name="st0", tag="sch")
                st1 = spool.tile([128, 2, 512], f32, name="st1", tag="sch")
                et0 = epool.tile([128, 1024], f16, name="et0", tag="et")
                et1 = epool.tile([128, 1024], f16, name="et1", tag="et")
                sa, sv = (st0, st1) if ci % 2 == 0 else (st1, st0)
                ea, ev = (et0, et1) if ci % 2 == 0 else (et1, et0)
                for j, m in enumerate(ms):
                    nc.tensor.matmul(st0[:, j, :],
                                     kT[0:64, 128 * m:128 * (m + 1)],
                                     qT[0:64, GQ * g:GQ * (g + 1)],
                                     start=True, stop=True)
                    nc.tensor.matmul(st1[:, j, :],
                                     kT[64:128, 128 * m:128 * (m + 1)],
                                     qT[64:128, GQ * g:GQ * (g + 1)],
                                     start=True, stop=True)
                nm = len(ms)
                w = 512 * nm
                nc.scalar.activation(ea[:, 0:w], sa[:, 0:nm, :],
                                     EXP, scale=SCALE)
                nc.vector.tensor_scalar(ev[:, 0:w].bitcast(i16),
                                        sv[:, 0:nm, :],
                                        A16, B16, MULT, ADD)
                ci += 1
                inflight.append((g, ms, et0, et1))
        while inflight:
            emit_av(*inflight.pop(0))

    nc.compile()
    return nc


def _get_nc():
    if "nc" not in _state:
        _state["nc"] = _build_nc()
    return _state["nc"]


def _make_in_maps(x, qkv_w, qkv_b):
    f16 = np.float16
    x = np.asarray(x, np.float32)
    qkv_w = np.asarray(qkv_w, np.float32)
    qkv_b = np.asarray(qkv_b, np.float32)
    in_maps = []
    for core in range(8):
        b, hp = divmod(core, 4)
        h0, h1 = 2 * hp, 2 * hp + 1
        xt = np.ascontiguousarray(x[b].T).astype(f16)
        rq = np.concatenate([qkv_w[64 * h0:64 * h0 + 64],
                             qkv_w[64 * h1:64 * h1 + 64]], 0)
        rk = np.concatenate([qkv_w[C + 64 * h0:C + 64 * h0 + 64],
                             qkv_w[C + 64 * h1:C + 64 * h1 + 64]], 0)
        wqk = np.ascontiguousarray(np.concatenate([rq, rk], 0).T).astype(f16)
        bq = np.concatenate([qkv_b[64 * h0:64 * h0 + 64],
                             qkv_b[64 * h1:64 * h1 + 64]])
        bk = np.concatenate([qkv_b[C + 64 * h0:C + 64 * h0 + 64],
                             qkv_b[C + 64 * h1:C + 64 * h1 + 64]])
        bqk = np.ascontiguousarray(np.stack([bq, bk], 1)).astype(np.float32)
        rv = np.concatenate([qkv_w[2 * C + 64 * h0:2 * C + 64 * h0 + 64],
                             qkv_w[2 * C + 64 * h1:2 * C + 64 * h1 + 64]], 0)
        wv = np.ascontiguousarray(rv.T).astype(f16)
        bvrow = np.concatenate([qkv_b[2 * C + 64 * h0:2 * C + 64 * h0 + 64],
                                qkv_b[2 * C + 64 * h1:2 * C + 64 * h1 + 64]])
        bv = np.ascontiguousarray(
            np.broadcast_to(bvrow[None, :], (128, 128))).astype(f16)
        in_maps.append(dict(xt=xt, wqk=wqk, bqk=bqk, wv=wv, bv=bv))
    return in_maps


def _gather(results, proj_w, proj_b):
    proj_w = np.asarray(proj_w, np.float32)
    proj_b = np.asarray(proj_b, np.float32)
    out = np.empty((B, N, C), np.float32)
    for b in range(B):
        Xb = np.empty((N, C), np.float32)
        for hp in range(4):
            av = results[4 * b + hp]["av"].astype(np.float32)
            av = av.reshape(NG, 2, 65, GQ)
            for lh in range(2):
                att = av[:, lh, 0:64, :] / av[:, lh, 64:65, :]  # [NG,64,GQ]
                att = att.transpose(0, 2, 1).reshape(N, 64)
                Xb[:, 128 * hp + 64 * lh:128 * hp + 64 * lh + 64] = att
        out[b] = Xb @ proj_w.T + proj_b
    return out


def _run(x, qkv_w, qkv_b, proj_w, proj_b, trace=False, tmpdir=None):
    from concourse import bass_utils
    nc = _get_nc()
    in_maps = _make_in_maps(x, qkv_w, qkv_b)
    res = bass_utils.run_bass_kernel_spmd(
        nc, in_maps, core_ids=list(range(8)), trace=trace, tmpdir=tmpdir)
    return _gather(res.results, proj_w, proj_b), res


def kernel(x, qkv_w, qkv_b, proj_w, proj_b):
    out, _ = _run(x, qkv_w, qkv_b, proj_w, proj_b, trace=False)
    return out
